# revision 1
# baseline (speedup 1.0000x reference)
"""Trainium2 Bass kernel for nn_CMAF (cross-modal attention fusion block).

Layout: feature-major activations on-chip — every tile is
[128 features (partitions) x 512 samples (free)], so all matmuls are
weight-stationary bf16 with the batch as the moving free dimension.
Inputs are pre-cast to bf16 host-side and loaded straight into
feature-major SBUF via DMA-transpose (2-byte xbar path), so no on-chip
input transposes are needed.

Cross-partition work (LayerNorm stats, softmax sums) is done with
ones-matrix matmuls that fuse the partition reduction AND the partition
broadcast into a single PE instruction.  The 2-way attention softmax
collapses to sigmoid((s0-s1)/sqrt(dh)), with s0-s1 accumulated in PSUM
by a +/- pair of block-diagonal head-mask matmuls.

LayerNorm mean subtraction is folded into the weights host-side
(centering matrix C = I - 11^T/128 on each producing linear layer);
the residual-stream means are zero by construction given the
(asserted) unit/zero LN affine params.

Data parallel over 8 NeuronCores: 8192 samples each.
"""

import numpy as np
import ml_dtypes

import concourse.bass as bass
import concourse.mybir as mybir
from concourse.tile import TileContext
from concourse.vector_clock import ScopedClock
from concourse.bass_utils import run_bass_kernel_spmd

F32 = mybir.dt.float32
BF16 = mybir.dt.bfloat16
AL = mybir.AluOpType
AF = mybir.ActivationFunctionType
NPBF = ml_dtypes.bfloat16

D = 128
SP = 1280
FFN = 256
NB = 3
DH = 32
KV_IDX = ((1, 2), (0, 2), (0, 1))
NCORES = 8
BLK = 1024
MMN = 512
EPS = 1e-5
ISQ = float(1.0 / np.sqrt(DH))


def _patch_tile_drain():
    """walrus here rejects >4 sem waits on one instruction; Tile's tail
    drain carries one wait per logical proc.  Re-emit them as standalone
    wait_ge instructions ahead of the drain."""
    TC = TileContext
    if getattr(TC, "_drain_patched", False):
        return

    def patched(self, tick_clock, wait_clock):
        nop_inst = self.nc.sync.nop()
        wait_clock.add_sem_waits(
            nop_inst.ins, ScopedClock({None: tick_clock.global_clock})
        )
        d = nop_inst.ins
        si = d.sync_info
        waits = list(si.on_wait) if si is not None else []
        if len(waits) > 4:
            si.on_wait = []
            d.sync_info = si
            name2sem = {s.name: s for s in self.sems.allocated().values()}
            for w in waits:
                sem = name2sem.get(w.ant_name)
                if sem is None:
                    raise RuntimeError(f"drain patch: unknown sem {w.ant_name}")
                self.nc.sync.wait_ge(sem, w.wait_value)
        self.nc.sync.drain()
        self.nc.all_engine_barrier()
        popped = self.nc._tile_sem_poison_stack.pop()
        assert popped is self._sem_poison
        self.nc.clear_and_free_semaphores(list(self.sems.allocated().values()))
        self.nc.all_engine_barrier()

    TC._drain_and_barrier = patched
    TC._drain_patched = True


def _fix_wait_overflow(nc):
    """walrus enforces per-opcode caps on sync-wait commands attached to
    one instruction (DmaTransposeAnt: 1, others: ~4).  Move the excess
    onto same-engine NOPs inserted immediately before the instruction."""
    LIMITS = {}
    DEFAULT_LIM = 1
    for fn in nc.m.functions:
        for bb in fn.blocks:
            insts = list(bb.instructions)
            out = []
            changed = False
            for inst in insts:
                si = getattr(inst, "sync_info", None)
                w = list(si.on_wait) if si is not None and si.on_wait else []
                lim = LIMITS.get(type(inst).__name__, DEFAULT_LIM)
                if len(w) > lim:
                    excess = w[lim:]
                    keep = w[:lim]
                    eng = nc.engines[inst.engine]
                    nops = []
                    for i in range(0, len(excess), 1):
                        chunk = excess[i:i + 1]
                        nop_bi = eng.nop()
                        nop_inst = nop_bi.ins
                        cb = nc.cur_bb.bb
                        cb.instructions = [x for x in cb.instructions
                                           if x.name != nop_inst.name]
                        import bass_rust
                        nop_inst.sync_info = bass_rust.SyncInfo(
                            on_wait=chunk, on_update=[])
                        nops.append(nop_inst)
                    si.on_wait = keep
                    inst.sync_info = si
                    out.extend(nops)
                    changed = True
                out.append(inst)
            if changed:
                bb.instructions = out


def prep_weights(inp):
    """Host-side prep of all weights into SBUF layouts. bf16 for matmul
    operands, fp32 for per-partition bias vectors."""
    f64 = np.float64
    C = np.eye(D, dtype=f64) - 1.0 / D

    def bf(a):
        return np.ascontiguousarray(a.astype(np.float32)).astype(NPBF)

    def f32(a):
        return np.ascontiguousarray(a, dtype=np.float32)

    w = {}
    wsp = C @ inp["proj_w_spatial"].astype(f64)            # [128,1280]
    w["wspT"] = bf(np.transpose(wsp.reshape(D, 10, D), (2, 1, 0)).reshape(D, 10 * D))
    wgf = np.stack([C @ inp["proj_w_gf"][i].astype(f64) for i in range(2)])
    w["wgfT"] = bf(np.transpose(wgf, (2, 0, 1)).reshape(D, 2 * D))
    w["bc"] = f32(C @ inp["proj_b"].astype(f64).T)         # [128,3]
    w["emb"] = f32(inp["mod_emb"].T)

    ipw = inp["in_proj_w"].astype(f64)                     # [3, 384, 128]
    wq, wk, wv = ipw[:, :D], ipw[:, D:2 * D], ipw[:, 2 * D:]
    w["wqT"] = bf(np.transpose(wq, (2, 0, 1)).reshape(D, NB * D))
    w["wkT"] = bf(np.transpose(wk, (2, 0, 1)).reshape(D, NB * D))
    w["wvT"] = bf(np.transpose(wv, (2, 0, 1)).reshape(D, NB * D))
    ow = np.stack([C @ inp["out_proj_w"][n].astype(f64) for n in range(NB)])
    w["owT"] = bf(np.transpose(ow, (2, 0, 1)).reshape(D, NB * D))
    ob2 = np.stack([
        C @ inp["out_proj_b"][n].astype(f64)
        - inp["mod_emb"][n].astype(f64).mean()
        for n in range(NB)])
    w["ob2"] = f32(ob2.T)

    w1 = inp["ffn_w1"].astype(f64)                         # [3, 256, 128]
    w["w1T"] = bf(np.transpose(w1, (2, 0, 1)).reshape(D, NB * FFN))
    w["b1"] = f32(inp["ffn_b1"].reshape(NB * 2, D).T)      # [128, 6]
    w2 = np.stack([C @ inp["ffn_w2"][n].astype(f64) for n in range(NB)])
    w2c = w2.reshape(NB, D, 2, D)                          # [n, j, c, p]
    w["w2T"] = bf(np.transpose(w2c, (3, 0, 2, 1)).reshape(D, NB * 2 * D))
    b2c = np.stack([C @ inp["ffn_b2"][n].astype(f64) for n in range(NB)])
    w["b2c"] = f32(b2c.T)

    gw = inp["gate_w"].astype(f64).reshape(NB, NB, D)      # [j, n, p]
    w["gwT"] = bf(np.transpose(gw, (2, 1, 0)).reshape(D, NB * NB))
    w["gateb"] = f32(inp["gate_b"].reshape(NB, 1))

    w["onesT"] = bf(np.full((D, D), 1.0 / D))
    hs = np.zeros((D, D), dtype=np.float32)
    for h in range(4):
        hs[h * DH:(h + 1) * DH, h * DH:(h + 1) * DH] = 1.0
    w["hsel"] = bf(hs)
    w["hseln"] = bf(-hs)
    w["ones3"] = bf(np.ones((NB, D)))
    esel = np.zeros((NB, NB * D), dtype=np.float32)
    for n in range(NB):
        esel[n, n * D:(n + 1) * D] = 1.0
    w["esel"] = bf(esel)
    w["ident"] = bf(np.eye(D))
    w["epsv"] = np.full((D, 1), EPS, dtype=np.float32)
    w["zerov"] = np.zeros((D, 1), dtype=np.float32)

    assert np.allclose(inp["proj_ln_g"], 1) and np.allclose(inp["proj_ln_b"], 0)
    assert np.allclose(inp["attn_ln_g"], 1) and np.allclose(inp["attn_ln_b"], 0)
    assert np.allclose(inp["ffn_ln_g"], 1) and np.allclose(inp["ffn_ln_b"], 0)
    assert np.allclose(inp["in_proj_b"], 0)
    return w


WEIGHT_SPECS = {
    "wspT": ((D, 10 * D), BF16), "wgfT": ((D, 2 * D), BF16),
    "bc": ((D, NB), F32), "emb": ((D, NB), F32),
    "wqT": ((D, NB * D), BF16), "wkT": ((D, NB * D), BF16),
    "wvT": ((D, NB * D), BF16), "owT": ((D, NB * D), BF16),
    "ob2": ((D, NB), F32),
    "w1T": ((D, NB * FFN), BF16), "b1": ((D, NB * 2), F32),
    "w2T": ((D, NB * 2 * D), BF16), "b2c": ((D, NB), F32),
    "gwT": ((D, NB * NB), BF16), "gateb": ((NB, 1), F32),
    "onesT": ((D, D), BF16), "hsel": ((D, D), BF16), "hseln": ((D, D), BF16),
    "ones3": ((NB, D), BF16), "esel": ((NB, NB * D), BF16),
    "ident": ((D, D), BF16),
    "epsv": ((D, 1), F32), "zerov": ((D, 1), F32),
}


def build_program(Bc, repeat=1):
    nc = bass.Bass()
    xsp = nc.dram_tensor("x_spatial", [Bc, SP], BF16, kind="ExternalInput")
    xg = nc.dram_tensor("x_gradient", [Bc, D], BF16, kind="ExternalInput")
    xf = nc.dram_tensor("x_frequency", [Bc, D], BF16, kind="ExternalInput")
    wd = {k: nc.dram_tensor(k, list(s[0]), s[1], kind="ExternalInput")
          for k, s in WEIGHT_SPECS.items()}
    out = nc.dram_tensor("out", [Bc, D], F32, kind="ExternalOutput")

    nblk = Bc // BLK
    assert Bc % BLK == 0

    with TileContext(nc) as tc, nc.allow_low_precision(reason="bf16 kernel"):
        with (
            tc.tile_pool(name="wp", bufs=1) as wp,
            tc.tile_pool(name="xin", bufs=2) as xin,
            tc.tile_pool(name="work", bufs=2) as wk_,
            tc.tile_pool(name="outp", bufs=1) as outp,
            tc.tile_pool(name="ps", bufs=4, space="PSUM") as psp,
        ):
            W = {}
            for k, s in WEIGHT_SPECS.items():
                W[k] = wp.tile(list(s[0]), s[1], tag=k, name=k)
                nc.gpsimd.dma_start(W[k][:], wd[k][:])
            ident = W["ident"]

            def mm(out_ap, lhsT, rhs, start=True, stop=True):
                for h in range(BLK // MMN):
                    nc.tensor.matmul(out_ap[:, h * MMN:(h + 1) * MMN], lhsT,
                                     rhs[:, h * MMN:(h + 1) * MMN],
                                     start=start, stop=stop)

            def phase0(b):
                r0 = (b % nblk) * BLK
                st = {}
                xspT_all = xin.tile([D, 10 * BLK], BF16, tag="xspT")
                nc.sync.dma_start(
                    xspT_all[:].rearrange("p (c n) -> p c n", c=10),
                    xsp[r0:r0 + BLK, :], transpose=True)
                st["xspT"] = xspT_all
                st["xgT"] = xin.tile([D, BLK], BF16, tag="xgT", name="xgT")
                nc.sync.dma_start(st["xgT"][:], xg[r0:r0 + BLK, :], transpose=True)
                st["xfT"] = xin.tile([D, BLK], BF16, tag="xfT", name="xfT")
                nc.sync.dma_start(st["xfT"][:], xf[r0:r0 + BLK, :], transpose=True)
                return st

            def ln_rb_into(sq_sb, rb):
                mq = psp.tile([D, BLK], F32, tag="ps")
                mm(mq[:], W["onesT"][:], sq_sb[:])
                lnv = wk_.tile([D, BLK], F32, tag="lnv", bufs=1)
                nc.scalar.activation(lnv[:], mq[:], AF.Ln,
                                     bias=W["epsv"][:, 0:1])
                nc.scalar.activation(rb[:], lnv[:], AF.Exp, scale=-0.5,
                                     bias=W["zerov"][:, 0:1])

            def ln_rb(sq_sb, tag):
                # rsqrt(v+eps) = exp(-0.5*ln(v+eps)): stays in the
                # natural_log_exp table set (no ACT table switches)
                mq = psp.tile([D, BLK], F32, tag="ps")
                mm(mq[:], W["onesT"][:], sq_sb[:])
                lnv = wk_.tile([D, BLK], F32, tag="lnv", bufs=1)
                nc.scalar.activation(lnv[:], mq[:], AF.Ln,
                                     bias=W["epsv"][:, 0:1])
                rb = wk_.tile([D, BLK], BF16, tag=tag)
                nc.scalar.activation(rb[:], lnv[:], AF.Exp, scale=-0.5,
                                     bias=W["zerov"][:, 0:1])
                return rb

            def phase1(st):
                z_ps = []
                zs = psp.tile([D, BLK], F32, tag="ps")
                for c in range(10):
                    mm(zs[:], W["wspT"][:, c * D:(c + 1) * D],
                       st["xspT"][:, c * BLK:(c + 1) * BLK],
                       start=(c == 0), stop=(c == 9))
                z_ps.append(zs)
                for i, key in ((0, "xgT"), (1, "xfT")):
                    zt = psp.tile([D, BLK], F32, tag="ps")
                    mm(zt[:], W["wgfT"][:, i * D:(i + 1) * D], st[key][:])
                    z_ps.append(zt)
                P = []
                for n in range(NB):
                    z_sb = wk_.tile([D, BLK], BF16, tag=f"zsb{n}", bufs=1)
                    nc.scalar.activation(z_sb[:], z_ps[n][:], AF.Identity,
                                         bias=W["bc"][:, n:n + 1])
                    sq = wk_.tile([D, BLK], BF16, tag="sq", bufs=1)
                    nc.scalar.activation(sq[:], z_sb[:], AF.Square,
                                         bias=W["zerov"][:, 0:1])
                    rb = ln_rb(sq, "rb")
                    p_ = wk_.tile([D, BLK], BF16, tag=f"P{n}")
                    nc.vector.tensor_tensor(p_[:], z_sb[:], rb[:], AL.mult)
                    nc.vector.tensor_scalar_add(p_[:], p_[:], W["emb"][:, n:n + 1])
                    P.append(p_)
                st["P"] = P
                # dP[n] = P[kv0] - P[kv1]: k/v differences come from a single
                # matmul each (linearity), halving attention PSUM pressure
                dP = []
                for n in range(NB):
                    s0, s1 = KV_IDX[n]
                    dp = wk_.tile([D, BLK], BF16, tag=f"dP{n}", bufs=2)
                    nc.vector.tensor_tensor(dp[:], P[s0][:], P[s1][:], AL.subtract)
                    dP.append(dp)
                st["dP"] = dP

            def phase2(st):
                P = st["P"]
                dP = st["dP"]
                x1 = []
                for n in range(NB):
                    s0, s1 = KV_IDX[n]
                    q_ps = psp.tile([D, BLK], F32, tag="ps")
                    mm(q_ps[:], W["wqT"][:, n * D:(n + 1) * D], P[n][:])
                    dk_ps = psp.tile([D, BLK], F32, tag="ps")
                    mm(dk_ps[:], W["wkT"][:, n * D:(n + 1) * D], dP[n][:])
                    dv_ps = psp.tile([D, BLK], F32, tag="ps")
                    mm(dv_ps[:], W["wvT"][:, n * D:(n + 1) * D], dP[n][:])
                    v1_ps = psp.tile([D, BLK], F32, tag="ps")
                    mm(v1_ps[:], W["wvT"][:, n * D:(n + 1) * D], P[s1][:])

                    q_sb = wk_.tile([D, BLK], BF16, tag="qsb", bufs=1)
                    nc.scalar.activation(q_sb[:], q_ps[:], AF.Copy)
                    v1_sb = wk_.tile([D, BLK], BF16, tag="v1sb", bufs=1)
                    nc.scalar.activation(v1_sb[:], v1_ps[:], AF.Copy)
                    t0 = wk_.tile([D, BLK], BF16, tag="t0", bufs=1)
                    nc.vector.tensor_tensor(t0[:], q_sb[:], dk_ps[:], AL.mult)
                    d_ps = psp.tile([D, BLK], F32, tag="ps")
                    mm(d_ps[:], W["hsel"][:], t0[:])
                    # sigmoid(d*ISQ) = 1/(1+exp(-d*ISQ)) — ln_exp set only
                    ea = wk_.tile([D, BLK], BF16, tag="ea", bufs=1)
                    nc.scalar.activation(ea[:], d_ps[:], AF.Exp,
                                         bias=W["zerov"][:, 0:1], scale=-ISQ)
                    ea1 = wk_.tile([D, BLK], BF16, tag="ea1", bufs=1)
                    nc.vector.tensor_scalar_add(ea1[:], ea[:], 1.0)
                    a0 = wk_.tile([D, BLK], BF16, tag="a0", bufs=1)
                    nc.vector.reciprocal(a0[:], ea1[:])
                    tp = wk_.tile([D, BLK], BF16, tag="tp", bufs=1)
                    nc.vector.tensor_tensor(tp[:], a0[:], dv_ps[:], AL.mult)

                    o_ps = psp.tile([D, BLK], F32, tag="ps")
                    mm(o_ps[:], W["owT"][:, n * D:(n + 1) * D], tp[:],
                       start=True, stop=False)
                    mm(o_ps[:], W["owT"][:, n * D:(n + 1) * D], v1_sb[:],
                       start=False, stop=True)
                    u = wk_.tile([D, BLK], BF16, tag=f"u{n}")
                    nc.vector.scalar_tensor_tensor(
                        u[:], o_ps[:], W["ob2"][:, n:n + 1], P[n][:],
                        AL.add, AL.add)
                    sq = wk_.tile([D, BLK], BF16, tag="sq", bufs=1)
                    nc.scalar.activation(sq[:], u[:], AF.Square,
                                         bias=W["zerov"][:, 0:1])
                    rb = wk_.tile([D, BLK], BF16, tag=f"rb2_{n}")
                    ln_rb_into(sq, rb)
                    x1n = wk_.tile([D, BLK], BF16, tag=f"x1{n}")
                    nc.vector.tensor_tensor(x1n[:], u[:], rb[:], AL.mult)
                    x1.append(x1n)
                    st.setdefault("u", []).append(u)
                    st.setdefault("rb2", []).append(rb)
                st["x1"] = x1

            def phase3a(st):
                u, rb2 = st["u"], st["rb2"]
                hs_all = []
                for n in range(NB):
                    h_sb = []
                    for c in range(2):
                        h_ps = psp.tile([D, BLK], F32, tag="ps")
                        mm(h_ps[:],
                           W["w1T"][:, n * FFN + c * D: n * FFN + (c + 1) * D],
                           u[n][:])
                        hpre = wk_.tile([D, BLK], BF16, tag=f"hpre{c}", bufs=1)
                        nc.vector.tensor_tensor(hpre[:], rb2[n][:], h_ps[:],
                                                AL.mult)
                        hs_ = wk_.tile([D, BLK], BF16, tag=f"hsb{n}_{c}", bufs=1)
                        nc.scalar.activation(hs_[:], hpre[:], AF.Gelu,
                                             bias=W["b1"][:, 2 * n + c: 2 * n + c + 1])
                        h_sb.append(hs_)
                    hs_all.append(h_sb)
                st["hs"] = hs_all

            def phase3b(st):
                x1 = st["x1"]
                x2 = []
                for n in range(NB):
                    h_sb = st["hs"][n]
                    f_ps = psp.tile([D, BLK], F32, tag="ps")
                    for c in range(2):
                        mm(f_ps[:], W["w2T"][:, (2 * n + c) * D:(2 * n + c + 1) * D],
                           h_sb[c][:], start=(c == 0), stop=(c == 1))
                    x2p = wk_.tile([D, BLK], BF16, tag="x2p", bufs=1)
                    nc.vector.scalar_tensor_tensor(
                        x2p[:], f_ps[:], W["b2c"][:, n:n + 1], x1[n][:],
                        AL.add, AL.add)
                    sq = wk_.tile([D, BLK], BF16, tag="sq", bufs=1)
                    nc.scalar.activation(sq[:], x2p[:], AF.Square,
                                         bias=W["zerov"][:, 0:1])
                    rb = ln_rb(sq, "rb")
                    x2n = wk_.tile([D, BLK], BF16, tag=f"x2{n}")
                    nc.vector.tensor_tensor(x2n[:], x2p[:], rb[:], AL.mult)
                    x2.append(x2n)
                st["x2"] = x2

            def phase4a(st):
                x2 = st["x2"]
                g_ps = psp.tile([NB, BLK], F32, tag="ps")
                for n in range(NB):
                    mm(g_ps[:], W["gwT"][:, n * NB:(n + 1) * NB], x2[n][:],
                       start=(n == 0), stop=(n == 2))
                e_sb = wk_.tile([NB, BLK], BF16, tag="esb", bufs=1)
                nc.scalar.activation(e_sb[:], g_ps[:], AF.Exp,
                                     bias=W["gateb"][:NB, 0:1])
                zb_ps = psp.tile([D, BLK], F32, tag="ps")
                mm(zb_ps[:], W["ones3"][:NB, :], e_sb[:])
                rz = wk_.tile([D, BLK], BF16, tag="rz", bufs=1)
                nc.vector.reciprocal(rz[:], zb_ps[:])
                mns = []
                for n in range(NB):
                    eb_ps = psp.tile([D, BLK], F32, tag="ps")
                    mm(eb_ps[:], W["esel"][:NB, n * D:(n + 1) * D], e_sb[:])
                    mn = wk_.tile([D, BLK], BF16, tag=f"mn{n}", bufs=2)
                    nc.vector.tensor_tensor(mn[:], x2[n][:], eb_ps[:], AL.mult)
                    mns.append(mn)
                st["mn"] = mns
                st["rz"] = rz

            def phase4b(st, b):
                r0 = (b % nblk) * BLK
                mns, rz = st["mn"], st["rz"]
                acc = wk_.tile([D, BLK], BF16, tag="macc", bufs=1)
                nc.vector.tensor_tensor(acc[:], mns[0][:], mns[1][:], AL.add)
                acc2 = wk_.tile([D, BLK], BF16, tag="macc2", bufs=1)
                nc.vector.tensor_tensor(acc2[:], acc[:], mns[2][:], AL.add)
                fused = wk_.tile([D, BLK], BF16, tag="fused", bufs=1)
                nc.vector.tensor_tensor(fused[:], acc2[:], rz[:], AL.mult)

                ob_sb = outp.tile([D, (BLK // D) * D], F32, tag="ob")
                for j in range(BLK // D):
                    ob_ps = psp.tile([D, D], BF16, tag="ps")
                    nc.tensor.matmul(ob_ps[:], fused[:, j * D:(j + 1) * D],
                                     ident[:], is_transpose=True)
                    nc.vector.tensor_copy(ob_sb[:, j * D:(j + 1) * D], ob_ps[:])
                nc.gpsimd.dma_start(
                    out[r0:r0 + BLK, :].rearrange("(j p) k -> p j k", p=D),
                    ob_sb[:].rearrange("p (j k) -> p j k", j=BLK // D))

            # software-pipelined emission; gelu ops grouped at tick head so
            # the ACT table set switches at most twice per tick
            total = nblk * repeat
            bstate = {}
            for t in range(total + 4):
                if 0 <= t - 3 < total:
                    phase3a(bstate[t - 3])
                if 0 <= t - 4 < total:
                    phase4a(bstate[t - 4])
                if 0 <= t - 2 < total:
                    phase2(bstate[t - 2])
                if 0 <= t - 1 < total:
                    phase1(bstate[t - 1])
                if 0 <= t - 3 < total:
                    phase3b(bstate[t - 3])
                if 0 <= t - 4 < total:
                    phase4b(bstate.pop(t - 4), t - 4)
                if t < total:
                    bstate[t] = phase0(t)
    _fix_wait_overflow(nc)
    return nc


def kernel(**inputs):
    _patch_tile_drain()
    B = inputs["x_spatial"].shape[0]
    Bc = B // NCORES
    w = prep_weights(inputs)
    nc = build_program(Bc)
    xb = {k: np.ascontiguousarray(inputs[k]).astype(NPBF)
          for k in ("x_spatial", "x_gradient", "x_frequency")}
    in_maps = []
    for c in range(NCORES):
        m = dict(w)
        for k in ("x_spatial", "x_gradient", "x_frequency"):
            m[k] = np.ascontiguousarray(xb[k][c * Bc:(c + 1) * Bc])
        in_maps.append(m)
    res = run_bass_kernel_spmd(nc, in_maps, list(range(NCORES)))
    return np.concatenate([res.results[c]["out"] for c in range(NCORES)], axis=0)

